# revision 12
# baseline (speedup 1.0000x reference)
"""HGNN layer on 8 Trainium2 NeuronCores (Bass/Tile).

Reference computation:
    x1 = x @ W1                                    [N, F]
    w = softmax(where(seq > 0, 1, -9e15))          uniform over valid slots
    edge = relu(sum_l w[e,l] * x1[seq[e,l]])       [E, F]
    e1 = edge @ W2                                 [E, F]
    uw = softmax(where(useq > 0, 1, -9e15))
    node = sum_l uw[n,l] * e1[useq[n,l]]           [N, F]

Strategy (8-way SPMD), v5:
  - linearity: mean_l x1[seq[e,l]] == (mean_l x[seq[e,l]]) @ W1, so gather RAW
    x rows (replicated fp16 table as input on every core) and matmul AFTER the
    reduce -> no stage-0 matmul over N rows and no x1 AllGather at all.
  - gathers are 128-row single-offset indirect DMAs (the only HW-supported
    form) on a single SWDGE queue; all offsets are preloaded into SBUF in
    one DMA so the Pool sequencer never stalls on index loads.
  - fp16 gather tables (x and e1) halve DMA traffic; tree-reduce in fp16,
    matmuls in fp32.
  - uniform-softmax weights (1/count) and the all-padding fallback (row 0)
    are precomputed HOST-side: counts become a per-row reciprocal input and
    empty rows get slot 0 remapped to row 0 with rec=1.
  - stage 1: gather x rows per edge, tree-reduce, scale, @W1, relu, @W2
    -> e1 shard (fp16) -> AllGather e1 table.
  - stage 2: gather e1 rows per node from the table, reduce, scale -> out.
"""

import sys

sys.path.insert(0, "/opt/trn_rl_repo")

import numpy as np

N = 50000
E = 25000
F = 256
L = 32
P = 128
NC_COUNT = 8
NSH = N // NC_COUNT        # 6250 nodes per core
ESH = E // NC_COUNT        # 3125 edges per core
NSH_PAD = 6272             # 49 tiles
ESH_PAD = 3200             # 25 tiles
N_TILES_NODE = NSH_PAD // P
N_TILES_EDGE = ESH_PAD // P
XZERO = N                  # zero-row index in the x table
EZERO = ESH                # zero row of shard 0 in the e1 table


def _derive():
    pass

NQ = 1                     # SWDGE queues to round-robin gathers over


def build_program():
    from concourse import bacc, bass, mybir, tile  # noqa: F401

    fp32 = mybir.dt.float32
    fp16 = mybir.dt.float16
    i32 = mybir.dt.int32

    nc = bacc.Bacc("TRN2", target_bir_lowering=False, debug=False,
                   num_devices=NC_COUNT, num_swdge_queues=NQ)

    x16 = nc.dram_tensor("x16", [N + 1, F], fp16, kind="ExternalInput").ap()
    w1 = nc.dram_tensor("w1", [F, F], fp32, kind="ExternalInput").ap()
    w2 = nc.dram_tensor("w2", [F, F], fp32, kind="ExternalInput").ap()
    identin = nc.dram_tensor("identin", [P, P], fp32,
                             kind="ExternalInput").ap()
    seqT = nc.dram_tensor("seqT", [P, N_TILES_EDGE * L], i32,
                          kind="ExternalInput").ap()
    useqT = nc.dram_tensor("useqT", [P, N_TILES_NODE * L], i32,
                           kind="ExternalInput").ap()
    rece = nc.dram_tensor("rece", [P, N_TILES_EDGE], fp32,
                          kind="ExternalInput").ap()
    recn = nc.dram_tensor("recn", [P, N_TILES_NODE], fp32,
                          kind="ExternalInput").ap()
    out = nc.dram_tensor("out", [NSH_PAD, F], fp32, kind="ExternalOutput").ap()

    AL = mybir.AluOpType

    with tile.TileContext(nc) as tc:
        with (
            tc.tile_pool(name="cst", bufs=1) as cst,
            tc.tile_pool(name="sb", bufs=4) as sbp,
            tc.tile_pool(name="gb", bufs=4) as gbp,
            tc.tile_pool(name="ps", bufs=2, space="PSUM") as psp,
            tc.tile_pool(name="pst", bufs=2, space="PSUM") as pstp,
            tc.tile_pool(name="dram", bufs=1, space="DRAM") as dram,
        ):
            # ---------- constants ----------
            ident = cst.tile([P, P], fp32)
            nc.sync.dma_start(out=ident[:], in_=identin[:, :])
            w1sb = [cst.tile([P, F], fp32, name=f"w1k{k}") for k in range(2)]
            w2sb = [cst.tile([P, F], fp32, name=f"w2k{k}") for k in range(2)]
            for k in range(2):
                nc.sync.dma_start(out=w1sb[k][:], in_=w1[k * P:(k + 1) * P, :])
                nc.sync.dma_start(out=w2sb[k][:], in_=w2[k * P:(k + 1) * P, :])
            zrow16 = cst.tile([1, F], fp16)
            nc.vector.memset(zrow16[:], 0.0)
            recesb = cst.tile([P, N_TILES_EDGE], fp32)
            nc.sync.dma_start(out=recesb[:], in_=rece[:, :])
            recnsb = cst.tile([P, N_TILES_NODE], fp32)
            nc.sync.dma_start(out=recnsb[:], in_=recn[:, :])
            seqsb = cst.tile([P, N_TILES_EDGE * L], i32)
            nc.sync.dma_start(out=seqsb[:], in_=seqT[:, :])
            useqsb = cst.tile([P, N_TILES_NODE * L], i32)
            nc.sync.dma_start(out=useqsb[:], in_=useqT[:, :])

            # ---------- DRAM scratch ----------
            # shard carries a trailing zero row so the AllGather table has a
            # zero row per shard without a second writer on the Shared tensor
            e1loc = dram.tile([ESH + 1, F], fp16)
            e1tab = dram.tile([NC_COUNT * (ESH + 1), F], fp16,
                              addr_space="Shared")
            nc.sync.dma_start(out=e1loc[ESH:ESH + 1, :], in_=zrow16[:])

            def gather_reduce(idxsb, t, table, recsb):
                """One 128-row tile: 32 single-offset gathers (offsets
                preloaded in SBUF), in-place halving-tree reduce (fp16),
                host-precomputed 1/count scale. Returns agg [P, F] fp32."""
                g = gbp.tile([P, L, F], fp16, tag="gb")
                for l in range(L):
                    c = t * L + l
                    inst = nc.gpsimd.indirect_dma_start(
                        out=g[:, l, :],
                        out_offset=None,
                        in_=table[:],
                        in_offset=bass.IndirectOffsetOnAxis(
                            ap=idxsb[:, c:c + 1], axis=0),
                    )
                    q = l % NQ
                    if q:
                        inst.ins.queue = f"qPoolDynamic{q}"
                h = L
                while h > 1:
                    h //= 2
                    nc.vector.tensor_tensor(
                        out=g[:, 0:h, :], in0=g[:, 0:h, :],
                        in1=g[:, h:2 * h, :], op=AL.add)
                agg = sbp.tile([P, F], fp32, tag="agg")
                nc.vector.tensor_scalar(
                    out=agg[:], in0=g[:, 0, :],
                    scalar1=recsb[:, t:t + 1], scalar2=None, op0=AL.mult)
                return agg

            def mm_rows(src, wsb):
                """src [P, F] fp32 @ W[F, F] -> psum [P, F] fp32."""
                ps = psp.tile([P, F], fp32, tag="mm")
                for kc in range(2):
                    pst = pstp.tile([P, P], fp32, tag="tr")
                    nc.tensor.transpose(
                        out=pst[:], in_=src[:, kc * P:(kc + 1) * P],
                        identity=ident[:])
                    srcT = sbp.tile([P, P], fp32, tag="srcT")
                    nc.vector.tensor_copy(out=srcT[:], in_=pst[:])
                    nc.tensor.matmul(ps[:], srcT[:], wsb[kc][:],
                                     start=(kc == 0), stop=(kc == 1))
                return ps

            # ---------- stage 1: edges ----------
            with nc.named_scope("stage1"):
                for t in range(N_TILES_EDGE):
                    agg = gather_reduce(seqsb, t, x16, recesb)
                    ps1 = mm_rows(agg, w1sb)
                    edge = sbp.tile([P, F], fp32, tag="edge")
                    nc.vector.tensor_scalar(
                        out=edge[:], in0=ps1[:], scalar1=0.0, scalar2=None,
                        op0=AL.max)  # relu
                    ps2 = mm_rows(edge, w2sb)
                    pr = min(P, ESH - t * P)
                    if pr > 0:
                        e1sb = sbp.tile([P, F], fp16, tag="row")
                        nc.vector.tensor_copy(out=e1sb[:pr, :], in_=ps2[:pr, :])
                        nc.sync.dma_start(out=e1loc[t * P:t * P + pr, :],
                                          in_=e1sb[:pr, :])
                nc.gpsimd.collective_compute(
                    "AllGather", AL.bypass,
                    replica_groups=[list(range(NC_COUNT))],
                    ins=[e1loc.opt()], outs=[e1tab.opt()],
                )

            # ---------- stage 2: nodes ----------
            with nc.named_scope("stage2"):
                for t in range(N_TILES_NODE):
                    agg = gather_reduce(useqsb, t, e1tab, recnsb)
                    nc.sync.dma_start(out=out[t * P:(t + 1) * P, :],
                                      in_=agg[:])

    nc.compile()
    return nc


def make_in_maps(x, seq, useq, W1, W2):
    x = np.asarray(x, dtype=np.float32)
    W1 = np.asarray(W1, dtype=np.float32)
    W2 = np.asarray(W2, dtype=np.float32)
    seq = np.asarray(seq).astype(np.int64)
    useq = np.asarray(useq).astype(np.int64)

    x16 = np.zeros((N + 1, F), np.float16)
    x16[:N] = x.astype(np.float16)

    valid = seq > 0
    cnt_e = valid.sum(1)
    sm = np.where(valid, seq, XZERO)
    empty_e = cnt_e == 0
    sm[empty_e, 0] = 0  # all-padding edge -> x[0]
    rec_e = (1.0 / np.maximum(cnt_e, 1)).astype(np.float32)
    rec_e[empty_e] = 1.0

    uvalid = useq > 0
    cnt_n = uvalid.sum(1)
    um = np.where(uvalid, (useq // ESH) * (ESH + 1) + useq % ESH, ESH)
    empty_n = cnt_n == 0
    um[empty_n, 0] = 0  # e1 table row 0 is global edge 0 -> e1[0]
    rec_n = (1.0 / np.maximum(cnt_n, 1)).astype(np.float32)
    rec_n[empty_n] = 1.0

    ident = np.eye(P, dtype=np.float32)
    in_maps = []
    for c in range(NC_COUNT):
        seqp = np.full((ESH_PAD, L), XZERO, np.int32)
        seqp[:ESH] = sm[c * ESH:(c + 1) * ESH]
        useqp = np.full((NSH_PAD, L), ESH, np.int32)
        useqp[:NSH] = um[c * NSH:(c + 1) * NSH]
        # idxT[p, t*L + l] = idx[t*P + p, l]
        seqT = np.ascontiguousarray(
            seqp.reshape(N_TILES_EDGE, P, L).transpose(1, 0, 2).reshape(P, -1))
        useqT = np.ascontiguousarray(
            useqp.reshape(N_TILES_NODE, P, L).transpose(1, 0, 2).reshape(P, -1))

        rpe = np.ones(ESH_PAD, np.float32)
        rpe[:ESH] = rec_e[c * ESH:(c + 1) * ESH]
        rpn = np.ones(NSH_PAD, np.float32)
        rpn[:NSH] = rec_n[c * NSH:(c + 1) * NSH]
        # recT[p, t] = rec[t * P + p]
        receT = np.ascontiguousarray(rpe.reshape(N_TILES_EDGE, P).T)
        recnT = np.ascontiguousarray(rpn.reshape(N_TILES_NODE, P).T)

        in_maps.append({
            "x16": x16,
            "w1": W1,
            "w2": W2,
            "identin": ident,
            "seqT": seqT,
            "useqT": useqT,
            "rece": receT,
            "recn": recnT,
        })
    return in_maps


def kernel(x, seq, useq, W1, W2):
    from concourse.bass_utils import run_bass_kernel_spmd

    in_maps = make_in_maps(x, seq, useq, W1, W2)
    nc = build_program()
    res = run_bass_kernel_spmd(nc, in_maps, core_ids=list(range(NC_COUNT)),
                               trace=False)
    parts = [res.results[c]["out"][:NSH] for c in range(NC_COUNT)]
    return np.concatenate(parts, axis=0)


# revision 14
# speedup vs baseline: 1.0043x; 1.0043x over previous
"""HGNN layer on 8 Trainium2 NeuronCores (Bass/Tile).

Reference computation:
    x1 = x @ W1                                    [N, F]
    w = softmax(where(seq > 0, 1, -9e15))          uniform over valid slots
    edge = relu(sum_l w[e,l] * x1[seq[e,l]])       [E, F]
    e1 = edge @ W2                                 [E, F]
    uw = softmax(where(useq > 0, 1, -9e15))
    node = sum_l uw[n,l] * e1[useq[n,l]]           [N, F]

Strategy (8-way SPMD), v5:
  - linearity: mean_l x1[seq[e,l]] == (mean_l x[seq[e,l]]) @ W1, so gather RAW
    x rows (replicated fp16 table as input on every core) and matmul AFTER the
    reduce -> no stage-0 matmul over N rows and no x1 AllGather at all.
  - gathers are 128-row single-offset indirect DMAs (the only HW-supported
    form) on a single SWDGE queue; all offsets are preloaded into SBUF in
    one DMA so the Pool sequencer never stalls on index loads.
  - fp16 gather tables (x and e1) halve DMA traffic; tree-reduce in fp16,
    matmuls in fp32.
  - uniform-softmax weights (1/count) and the all-padding fallback (row 0)
    are precomputed HOST-side: counts become a per-row reciprocal input and
    empty rows get slot 0 remapped to row 0 with rec=1.
  - stage 1: gather x rows per edge, tree-reduce, scale, @W1, relu, @W2
    -> e1 shard (fp16) -> AllGather e1 table.
  - stage 2: gather e1 rows per node from the table, reduce, scale -> out.
"""

import sys

sys.path.insert(0, "/opt/trn_rl_repo")

import numpy as np

N = 50000
E = 25000
F = 256
L = 32
P = 128
NC_COUNT = 8
NSH = N // NC_COUNT        # 6250 nodes per core
ESH = E // NC_COUNT        # 3125 edges per core
NSH_PAD = 6272             # 49 tiles
ESH_PAD = 3200             # 25 tiles
N_TILES_NODE = NSH_PAD // P
N_TILES_EDGE = ESH_PAD // P
XZERO = N                  # zero-row index in the x table
EZERO = ESH                # zero row of shard 0 in the e1 table


def _derive():
    pass

NQ = 1                     # SWDGE queues to round-robin gathers over


def build_program():
    from concourse import bacc, bass, mybir, tile  # noqa: F401

    fp32 = mybir.dt.float32
    fp16 = mybir.dt.float16
    i32 = mybir.dt.int32

    nc = bacc.Bacc("TRN2", target_bir_lowering=False, debug=False,
                   num_devices=NC_COUNT, num_swdge_queues=NQ)

    x16 = nc.dram_tensor("x16", [N + 1, F], fp16, kind="ExternalInput").ap()
    w1 = nc.dram_tensor("w1", [F, F], fp32, kind="ExternalInput").ap()
    w2 = nc.dram_tensor("w2", [F, F], fp32, kind="ExternalInput").ap()
    identin = nc.dram_tensor("identin", [P, P], fp32,
                             kind="ExternalInput").ap()
    seqT = nc.dram_tensor("seqT", [P, N_TILES_EDGE * L], i32,
                          kind="ExternalInput").ap()
    useqT = nc.dram_tensor("useqT", [P, N_TILES_NODE * L], i32,
                           kind="ExternalInput").ap()
    rece = nc.dram_tensor("rece", [P, N_TILES_EDGE], fp32,
                          kind="ExternalInput").ap()
    recn = nc.dram_tensor("recn", [P, N_TILES_NODE], fp32,
                          kind="ExternalInput").ap()
    out = nc.dram_tensor("out", [NSH_PAD, F], fp32, kind="ExternalOutput").ap()

    AL = mybir.AluOpType

    with tile.TileContext(nc) as tc:
        with (
            tc.tile_pool(name="cst", bufs=1) as cst,
            tc.tile_pool(name="sb", bufs=3) as sbp,
            tc.tile_pool(name="gb", bufs=3) as gbp,
            tc.tile_pool(name="ps", bufs=2, space="PSUM") as psp,
            tc.tile_pool(name="pst", bufs=2, space="PSUM") as pstp,
            tc.tile_pool(name="dram", bufs=1, space="DRAM") as dram,
        ):
            # ---------- constants ----------
            ident = cst.tile([P, P], fp32)
            nc.sync.dma_start(out=ident[:], in_=identin[:, :])
            w1sb = [cst.tile([P, F], fp32, name=f"w1k{k}") for k in range(2)]
            w2sb = [cst.tile([P, F], fp32, name=f"w2k{k}") for k in range(2)]
            for k in range(2):
                nc.sync.dma_start(out=w1sb[k][:], in_=w1[k * P:(k + 1) * P, :])
                nc.sync.dma_start(out=w2sb[k][:], in_=w2[k * P:(k + 1) * P, :])
            zrow16 = cst.tile([1, F], fp16)
            nc.vector.memset(zrow16[:], 0.0)
            recesb = cst.tile([P, N_TILES_EDGE], fp32)
            nc.sync.dma_start(out=recesb[:], in_=rece[:, :])
            recnsb = cst.tile([P, N_TILES_NODE], fp32)
            nc.sync.dma_start(out=recnsb[:], in_=recn[:, :])
            seqsb = cst.tile([P, N_TILES_EDGE * L], i32)
            nc.sync.dma_start(out=seqsb[:], in_=seqT[:, :])
            useqsb = cst.tile([P, N_TILES_NODE * L], i32)
            nc.sync.dma_start(out=useqsb[:], in_=useqT[:, :])

            # ---------- DRAM scratch ----------
            # shard carries a trailing zero row so the AllGather table has a
            # zero row per shard without a second writer on the Shared tensor
            e1loc = dram.tile([ESH + 1, F], fp16)
            e1tab = dram.tile([NC_COUNT * (ESH + 1), F], fp16,
                              addr_space="Shared")

            def gather_reduce(idxsb, t, table, recsb):
                """One 128-row tile: 32 single-offset gathers (offsets
                preloaded in SBUF), in-place halving-tree reduce (fp16),
                host-precomputed 1/count scale. Returns agg [P, F] fp32."""
                g = gbp.tile([P, L, F], fp16, tag="gb")
                for l in range(L):
                    c = t * L + l
                    inst = nc.gpsimd.indirect_dma_start(
                        out=g[:, l, :],
                        out_offset=None,
                        in_=table[:],
                        in_offset=bass.IndirectOffsetOnAxis(
                            ap=idxsb[:, c:c + 1], axis=0),
                    )
                    q = l % NQ
                    if q:
                        inst.ins.queue = f"qPoolDynamic{q}"
                h = L
                while h > 1:
                    h //= 2
                    nc.vector.tensor_tensor(
                        out=g[:, 0:h, :], in0=g[:, 0:h, :],
                        in1=g[:, h:2 * h, :], op=AL.add)
                agg = sbp.tile([P, F], fp32, tag="agg")
                nc.vector.tensor_scalar(
                    out=agg[:], in0=g[:, 0, :],
                    scalar1=recsb[:, t:t + 1], scalar2=None, op0=AL.mult)
                return agg

            def mm_rows(src, wsb):
                """src [P, F] fp32 @ W[F, F] -> psum [P, F] fp32."""
                ps = psp.tile([P, F], fp32, tag="mm")
                for kc in range(2):
                    pst = pstp.tile([P, P], fp32, tag="tr")
                    nc.tensor.transpose(
                        out=pst[:], in_=src[:, kc * P:(kc + 1) * P],
                        identity=ident[:])
                    srcT = sbp.tile([P, P], fp32, tag="srcT")
                    nc.vector.tensor_copy(out=srcT[:], in_=pst[:])
                    nc.tensor.matmul(ps[:], srcT[:], wsb[kc][:],
                                     start=(kc == 0), stop=(kc == 1))
                return ps

            # ---------- stage 1: edges ----------
            with nc.named_scope("stage1"):
                for t in range(N_TILES_EDGE):
                    agg = gather_reduce(seqsb, t, x16, recesb)
                    ps1 = mm_rows(agg, w1sb)
                    edge = sbp.tile([P, F], fp32, tag="edge")
                    nc.vector.tensor_scalar(
                        out=edge[:], in0=ps1[:], scalar1=0.0, scalar2=None,
                        op0=AL.max)  # relu
                    ps2 = mm_rows(edge, w2sb)
                    pr = min(P, ESH - t * P)
                    if pr > 0:
                        e1sb = sbp.tile([P, F], fp16, tag="row")
                        nc.vector.tensor_copy(out=e1sb[:pr, :], in_=ps2[:pr, :])
                        nc.sync.dma_start(out=e1loc[t * P:t * P + pr, :],
                                          in_=e1sb[:pr, :])
                nc.sync.dma_start(out=e1loc[ESH:ESH + 1, :], in_=zrow16[:])
                nc.gpsimd.collective_compute(
                    "AllGather", AL.bypass,
                    replica_groups=[list(range(NC_COUNT))],
                    ins=[e1loc.opt()], outs=[e1tab.opt()],
                )

            # ---------- stage 2: nodes ----------
            with nc.named_scope("stage2"):
                for t in range(N_TILES_NODE):
                    agg = gather_reduce(useqsb, t, e1tab, recnsb)
                    nc.sync.dma_start(out=out[t * P:(t + 1) * P, :],
                                      in_=agg[:])

    nc.compile()
    return nc


def make_in_maps(x, seq, useq, W1, W2):
    x = np.asarray(x, dtype=np.float32)
    W1 = np.asarray(W1, dtype=np.float32)
    W2 = np.asarray(W2, dtype=np.float32)
    seq = np.asarray(seq).astype(np.int64)
    useq = np.asarray(useq).astype(np.int64)

    x16 = np.zeros((N + 1, F), np.float16)
    x16[:N] = x.astype(np.float16)

    valid = seq > 0
    cnt_e = valid.sum(1)
    sm = np.where(valid, seq, XZERO)
    empty_e = cnt_e == 0
    sm[empty_e, 0] = 0  # all-padding edge -> x[0]
    rec_e = (1.0 / np.maximum(cnt_e, 1)).astype(np.float32)
    rec_e[empty_e] = 1.0

    uvalid = useq > 0
    cnt_n = uvalid.sum(1)
    um = np.where(uvalid, (useq // ESH) * (ESH + 1) + useq % ESH, ESH)
    empty_n = cnt_n == 0
    um[empty_n, 0] = 0  # e1 table row 0 is global edge 0 -> e1[0]
    rec_n = (1.0 / np.maximum(cnt_n, 1)).astype(np.float32)
    rec_n[empty_n] = 1.0

    ident = np.eye(P, dtype=np.float32)
    in_maps = []
    for c in range(NC_COUNT):
        seqp = np.full((ESH_PAD, L), XZERO, np.int32)
        seqp[:ESH] = sm[c * ESH:(c + 1) * ESH]
        useqp = np.full((NSH_PAD, L), ESH, np.int32)
        useqp[:NSH] = um[c * NSH:(c + 1) * NSH]
        # idxT[p, t*L + l] = idx[t*P + p, l]
        seqT = np.ascontiguousarray(
            seqp.reshape(N_TILES_EDGE, P, L).transpose(1, 0, 2).reshape(P, -1))
        useqT = np.ascontiguousarray(
            useqp.reshape(N_TILES_NODE, P, L).transpose(1, 0, 2).reshape(P, -1))

        rpe = np.ones(ESH_PAD, np.float32)
        rpe[:ESH] = rec_e[c * ESH:(c + 1) * ESH]
        rpn = np.ones(NSH_PAD, np.float32)
        rpn[:NSH] = rec_n[c * NSH:(c + 1) * NSH]
        # recT[p, t] = rec[t * P + p]
        receT = np.ascontiguousarray(rpe.reshape(N_TILES_EDGE, P).T)
        recnT = np.ascontiguousarray(rpn.reshape(N_TILES_NODE, P).T)

        in_maps.append({
            "x16": x16,
            "w1": W1,
            "w2": W2,
            "identin": ident,
            "seqT": seqT,
            "useqT": useqT,
            "rece": receT,
            "recn": recnT,
        })
    return in_maps


def kernel(x, seq, useq, W1, W2):
    from concourse.bass_utils import run_bass_kernel_spmd

    in_maps = make_in_maps(x, seq, useq, W1, W2)
    nc = build_program()
    res = run_bass_kernel_spmd(nc, in_maps, core_ids=list(range(NC_COUNT)),
                               trace=False)
    parts = [res.results[c]["out"][:NSH] for c in range(NC_COUNT)]
    return np.concatenate(parts, axis=0)
